# revision 21
# baseline (speedup 1.0000x reference)
"""Trainium2 Bass kernel for nn_DiffusionDynamicInput.

Reference computation (per sample b):
    ctx  = wv_embs[b] + t_emb[b]                       (13, 1024)
    hid  = silu(ctx @ w1 + b1)                         (13, 512)
    wgen = (hid @ w2 + b2).reshape(13, 128, 9)         per-(band) 3x3 filters
    out[d,h,w] = sum_{n,dy,dx} wgen[n,d,(dy,dx)] * x[b,n,h+dy,w+dx]   (SAME pad)
    bias = (ctx @ wb + bb).sum(axis=0)                 (128,)
    out += bias[:, None, None]

Sharding: data-parallel over B=8 across the 8 NeuronCores (one sample per
core). Inside a core the dynamic conv runs as K=39 fp16 matmuls: partitions
hold (dy, n) pairs (dy materialized by shifted loads of a DRAM fp16 copy of
x), the dx shift is a free-dim offset into a 258-wide zero-padded row layout,
and the three dx matmuls accumulate into one PSUM bank. The hypernetwork runs
in fp32. The per-sample bias is fused into the PSUM->SBUF eviction.
"""

import numpy as np

import concourse.bacc as bacc
import concourse.bass as bass
import concourse.mybir as mybir
import concourse.tile as tile
from concourse.bass_utils import run_bass_kernel_spmd
from concourse.masks import make_identity

F32 = mybir.dt.float32
F16 = mybir.dt.float16

NB = 13          # bands
HH = WW = 256    # image
DE = 1024        # embed dim
DO = 128         # out channels
NCORES = 8

RBLK = 32        # image rows per X3 block
NBLK = HH // RBLK
WPAD = WW + 2    # 258: row layout with zero col at each end
GRP = 8          # psum banks cycled per half-block


def _build_bass(repeat: int = 1):
    # Bacc (not plain Bass): its finalize() runs generate_event_semaphores,
    # which splits multi-sem waits that TRN2 instruction structs can't hold.
    # repeat > 1 re-emits the main conv loop (benchmarking: slope between
    # repeat counts isolates device time from dispatch overhead).
    nc = bacc.Bacc(target_bir_lowering=False, debug=False)

    x_ext = nc.declare_dram_parameter("x", [NB, HH, WW], F32, isOutput=False)
    t_ext = nc.declare_dram_parameter("t_emb", [DE], F32, isOutput=False)
    wv_ext = nc.declare_dram_parameter("wv", [NB, DE], F32, isOutput=False)
    w1_ext = nc.declare_dram_parameter("w1", [DE, 4 * DO], F32, isOutput=False)
    b1_ext = nc.declare_dram_parameter("b1", [4 * DO], F32, isOutput=False)
    # w2p/b2p are host-permuted so generated-filter column c' = p*128 + d
    w2p_ext = nc.declare_dram_parameter("w2p", [4 * DO, DO * 9], F32, isOutput=False)
    b2p_ext = nc.declare_dram_parameter("b2p", [DO * 9], F32, isOutput=False)
    wb_ext = nc.declare_dram_parameter("wb", [DE, DO], F32, isOutput=False)
    bb_ext = nc.declare_dram_parameter("bb", [DO], F32, isOutput=False)
    out_ext = nc.declare_dram_parameter("out", [DO, HH, WW], F32, isOutput=True)

    with tile.TileContext(nc) as tc:
        with (
            tc.tile_pool(name="const", bufs=1) as const_pool,
            tc.tile_pool(name="dram", bufs=1, space="DRAM") as dram_pool,
            tc.tile_pool(name="hyp", bufs=1) as hyp_pool,
            tc.tile_pool(name="wstream", bufs=3) as wstream_pool,
            tc.tile_pool(name="castbuf", bufs=2) as cast_pool,
        ):
            # ---------------- phase 0: x (fp32) -> x16 (fp16) in DRAM -------
            # flat view: 13*256 = 3328 rows of 256; 128 partitions x 26 rows
            x16 = dram_pool.tile([NB * HH, WW], F16)
            x_wide = x_ext.ap().rearrange("n h w -> (n h) w").rearrange(
                "(p r) w -> p (r w)", p=128
            )
            x16_wide = x16[:].rearrange("(p r) w -> p (r w)", p=128)
            CHUNK = 26 * WW // 2  # half of each partition's rows
            for c in range(2):
                xs = cast_pool.tile([128, CHUNK], F32, tag="cast_in")
                xd = cast_pool.tile([128, CHUNK], F16, tag="cast_out")
                nc.sync.dma_start(xs[:], x_wide[:, c * CHUNK:(c + 1) * CHUNK])
                nc.vector.tensor_copy(xd[:], xs[:])
                nc.sync.dma_start(x16_wide[:, c * CHUNK:(c + 1) * CHUNK], xd[:])

            x16_3d = x16[:].rearrange("(n h) w -> n h w", n=NB)
            x16_flat = x16[:].rearrange("r w -> (r w)")

            # ---------------- hypernetwork (fp32) ---------------------------
            ident = const_pool.tile([128, 128], F32)
            make_identity(nc, ident[:])

            # t_emb as [128, 8] (e = k*128 + p)
            tT = hyp_pool.tile([128, 8], F32)
            nc.sync.dma_start(tT[:], t_ext.ap().rearrange("(k p) -> p k", p=128))
            b1T = hyp_pool.tile([128, 4], F32)
            nc.sync.dma_start(b1T[:], b1_ext.ap().rearrange("(m p) -> p m", p=128))
            bbT = hyp_pool.tile([128, 1], F32)
            nc.sync.dma_start(bbT[:], bb_ext.ap().rearrange("(p o) -> p o", o=1))
            b2pT = hyp_pool.tile([1, DO * 9], F32)
            nc.sync.dma_start(b2pT[:], b2p_ext.ap().rearrange("(o c) -> o c", o=1))
            ones1 = const_pool.tile([1, NB], F32)
            nc.vector.memset(ones1[:], 1.0)

            wv_t = hyp_pool.tile([NB, DE], F32)
            nc.sync.dma_start(wv_t[:], wv_ext.ap())

            # ctxT[e, k, n] = wv[n, k*128+e] + t[k*128+e]
            ctxT = hyp_pool.tile([128, 8, NB], F32)
            with tc.tile_pool(name="tp_psum", bufs=2, space="PSUM") as tp_psum:
                # warm-up op: absorbs the identity-producer (Pool) semaphore
                # into the PE engine clock so later transposes carry a single
                # wait (the fused LDW struct has one wait slot).
                ps_warm = tp_psum.tile([1, 1], F32, tag="warm", bufs=1)
                nc.tensor.transpose(ps_warm[:], ident[:1, :1], ident[:1, :1])
                for k in range(8):
                    ps = tp_psum.tile([128, NB], F32, tag="tp")
                    nc.tensor.transpose(
                        ps[:], wv_t[:, k * 128:(k + 1) * 128], ident[:NB, :NB]
                    )
                    nc.vector.tensor_scalar_add(ctxT[:, k, :], ps[:], tT[:, k:k + 1])

                # sT[e, k] = sum_n ctxT[e, k, n]
                sT = hyp_pool.tile([128, 8, 1], F32)
                nc.vector.reduce_sum(sT[:], ctxT[:], axis=mybir.AxisListType.X)

                # hidT[s, m, n] = silu(sum_e w1[e, m*128+s] * ctxT[e, n] + b1)
                hidT = hyp_pool.tile([128, 4, NB], F32)
                for m in range(4):
                    ps = tp_psum.tile([128, NB], F32, tag="hid")
                    for k in range(8):
                        w1t = wstream_pool.tile([128, 128], F32, tag="w1")
                        nc.sync.dma_start(
                            w1t[:],
                            w1_ext.ap()[
                                k * 128:(k + 1) * 128, m * 128:(m + 1) * 128
                            ],
                        )
                        nc.tensor.matmul(
                            ps[:], w1t[:], ctxT[:, k, :], start=(k == 0), stop=(k == 7)
                        )
                    nc.scalar.activation(
                        hidT[:, m, :], ps[:],
                        mybir.ActivationFunctionType.Silu, bias=b1T[:, m:m + 1],
                    )

                # wgen16[n, p*128+d] (fp16) = hid @ w2p + b2p
                wgen16 = hyp_pool.tile([NB, DO * 9], F16)
                for j in range(3):  # 1152 = 3 * 384
                    ps = tp_psum.tile([NB, 384], F32, tag="wgen")
                    for k in range(4):
                        w2t = wstream_pool.tile([128, 384], F32, tag="w2")
                        nc.sync.dma_start(
                            w2t[:],
                            w2p_ext.ap()[k * 128:(k + 1) * 128, j * 384:(j + 1) * 384],
                        )
                        nc.tensor.matmul(
                            ps[:], hidT[:, k, :], w2t[:], start=(k == 0), stop=False
                        )
                    nc.tensor.matmul(
                        ps[:], ones1[:], b2pT[:, j * 384:(j + 1) * 384],
                        start=False, stop=True,
                    )
                    nc.vector.tensor_copy(wgen16[:, j * 384:(j + 1) * 384], ps[:])

                # bias[d] = sum_e s[e] * wb[e, d] + 13 * bb[d]
                bb13 = hyp_pool.tile([128, 1], F32)
                nc.vector.tensor_scalar_mul(bb13[:], bbT[:], float(NB))
                ps_b = tp_psum.tile([128, 1], F32, tag="bias", bufs=1)
                for k in range(8):
                    wbt = wstream_pool.tile([128, 128], F32, tag="wb")
                    nc.sync.dma_start(
                        wbt[:], wb_ext.ap()[k * 128:(k + 1) * 128, :]
                    )
                    nc.tensor.matmul(
                        ps_b[:], wbt[:], sT[:, k, :], start=(k == 0), stop=(k == 7)
                    )
                bias_sb = hyp_pool.tile([128, 1], F32)
                nc.scalar.activation(
                    bias_sb[:], ps_b[:],
                    mybir.ActivationFunctionType.Identity, bias=bb13[:],
                )

            # X3/lhsT partition order is n-major: q = n*3 + dyi.
            # lhsT[dx][n*3+dyi, d] = wgen16[n, (dyi*3+dxi)*128 + d]
            lhsT = [
                hyp_pool.tile([3 * NB, DO], F16, tag=f"lhsT{i}", name=f"lhsT{i}")
                for i in range(3)
            ]
            # NOTE: only dim 0 of an SBUF AP crosses partitions, so the dest
            # needs one DMA per dy (partition stride 3, offset dyi).
            wgen16_4d = wgen16[:].rearrange("n (dy dx d) -> n dy dx d", dy=3, dx=3)
            for dxi in range(3):
                lhsT_g = lhsT[dxi][:].rearrange("(n dy) d -> n dy d", dy=3)
                for dyi in range(3):
                    nc.sync.dma_start(
                        lhsT_g[:, dyi, :],
                        wgen16_4d[:, dyi, dxi, :],
                    )

            # ---------------- main loop: dynamic conv -----------------------
            with (
                tc.tile_pool(name="x3", bufs=3) as x3_pool,
                tc.tile_pool(name="ostage", bufs=4) as ostage_pool,
                tc.tile_pool(name="cpsum", bufs=GRP, space="PSUM") as cpsum_pool,
            ):
                for blk in [b for _ in range(repeat) for b in range(NBLK)]:
                    y0 = blk * RBLK
                    x3 = x3_pool.tile([3 * NB, RBLK, WPAD], F16, tag="x3")
                    # zero the left/right pad columns
                    nc.gpsimd.memset(x3[:, :, 0:1], 0.0)
                    nc.gpsimd.memset(x3[:, :, WPAD - 1:WPAD], 0.0)
                    # edge blocks: zero the row that falls outside the image.
                    # Engine ops need 32-aligned partition bases, so zero the
                    # row across all 39 partitions; the in-range dy groups'
                    # DMAs rewrite it below.
                    if blk == 0:
                        nc.gpsimd.memset(x3[:, 0:1, :], 0.0)
                    if blk == NBLK - 1:
                        nc.gpsimd.memset(x3[:, RBLK - 1:RBLK, :], 0.0)
                    x3_g = x3[:].rearrange("(n dy) r c -> n dy r c", dy=3)
                    for dyi, dy in enumerate((-1, 0, 1)):
                        lo = max(0, -(y0 + dy))         # first valid dest row
                        hi = min(RBLK, HH - (y0 + dy))  # one past last valid
                        nc.sync.dma_start(
                            x3_g[:, dyi, lo:hi, 1:WW + 1],
                            x16_3d[:, y0 + dy + lo:y0 + dy + hi, :],
                        )
                    for half in range(RBLK // (2 * GRP)):
                        psums = [
                            cpsum_pool.tile(
                                [DO, 2, WW], F32, tag="cps", name=f"cps{g}"
                            )
                            for g in range(GRP)
                        ]
                        # dx order (0, -1, +1): the dx=0 matmul reads no pad
                        # columns, so it carries only the x3-DMA wait; the
                        # pad-memset (Pool) wait lands on the second matmul.
                        for step, dxi in enumerate((1, 0, 2)):
                            for g in range(GRP):
                                r0 = (half * GRP + g) * 2
                                nc.tensor.matmul(
                                    psums[g][:],
                                    lhsT[dxi][:],
                                    x3[:, r0:r0 + 2, dxi:dxi + WW],
                                    start=(step == 0),
                                    stop=(step == 2),
                                )
                        for g in range(GRP):
                            r0 = (half * GRP + g) * 2
                            ost = ostage_pool.tile([DO, 2, WW], F32, tag="ost")
                            if g % 2 == 0:
                                nc.scalar.activation(
                                    ost[:], psums[g][:],
                                    mybir.ActivationFunctionType.Identity,
                                    bias=bias_sb[:],
                                )
                            else:
                                nc.vector.tensor_scalar_add(
                                    ost[:], psums[g][:], bias_sb[:]
                                )
                            nc.sync.dma_start(
                                out_ext.ap()[:, y0 + r0:y0 + r0 + 2, :], ost[:]
                            )
    if not nc.is_finalized():
        nc.finalize()
    return nc


_NC_CACHE = None


def _get_bass():
    global _NC_CACHE
    if _NC_CACHE is None:
        _NC_CACHE = _build_bass()
    return _NC_CACHE


def kernel(**inputs) -> np.ndarray:
    x = np.ascontiguousarray(np.asarray(inputs["x"], dtype=np.float32))
    t_emb = np.ascontiguousarray(np.asarray(inputs["t_emb"], dtype=np.float32))
    wv = np.ascontiguousarray(np.asarray(inputs["wv_embs"], dtype=np.float32))
    w1 = np.ascontiguousarray(np.asarray(inputs["w1"], dtype=np.float32))
    b1 = np.ascontiguousarray(np.asarray(inputs["b1"], dtype=np.float32))
    w2 = np.asarray(inputs["w2"], dtype=np.float32)
    b2 = np.asarray(inputs["b2"], dtype=np.float32)
    wb = np.ascontiguousarray(np.asarray(inputs["wb"], dtype=np.float32))
    bb = np.ascontiguousarray(np.asarray(inputs["bb"], dtype=np.float32))

    # permute filter columns: c = d*9 + p  ->  c' = p*128 + d
    w2p = np.ascontiguousarray(
        w2.reshape(4 * DO, DO, 9).transpose(0, 2, 1).reshape(4 * DO, DO * 9)
    )
    b2p = np.ascontiguousarray(b2.reshape(DO, 9).T.reshape(DO * 9))

    nc = _get_bass()
    in_maps = [
        {
            "x": x[b], "t_emb": t_emb[b], "wv": wv[b],
            "w1": w1, "b1": b1, "w2p": w2p, "b2p": b2p, "wb": wb, "bb": bb,
        }
        for b in range(NCORES)
    ]
    res = run_bass_kernel_spmd(nc, in_maps, list(range(NCORES)))
    return np.stack([res.results[b]["out"] for b in range(NCORES)], axis=0)


if __name__ == "__main__":
    rng = np.random.default_rng(0)
    demo = {
        "x": rng.standard_normal((NCORES, NB, HH, WW), dtype=np.float32),
        "t_emb": rng.standard_normal((NCORES, DE), dtype=np.float32),
        "wv_embs": rng.standard_normal((NCORES, NB, DE), dtype=np.float32),
        "w1": rng.standard_normal((DE, 4 * DO), dtype=np.float32) * 0.02,
        "b1": np.zeros(4 * DO, np.float32),
        "w2": rng.standard_normal((4 * DO, DO * 9), dtype=np.float32) * 0.02,
        "b2": np.zeros(DO * 9, np.float32),
        "wb": rng.standard_normal((DE, DO), dtype=np.float32) * 0.02,
        "bb": np.zeros(DO, np.float32),
    }
    out = kernel(**demo)
    print("out", out.shape, out.dtype, float(np.abs(out).mean()))


# revision 30
# speedup vs baseline: 4.2839x; 4.2839x over previous
"""Trainium2 Bass kernel for nn_DiffusionDynamicInput.

Reference computation (per sample b):
    ctx  = wv_embs[b] + t_emb[b]                       (13, 1024)
    hid  = silu(ctx @ w1 + b1)                         (13, 512)
    wgen = (hid @ w2 + b2).reshape(13, 128, 9)         per-(band) 3x3 filters
    out[d,h,w] = sum_{n,dy,dx} wgen[n,d,(dy,dx)] * x[b,n,h+dy,w+dx]   (SAME pad)
    bias = (ctx @ wb + bb).sum(axis=0)                 (128,)
    out += bias[:, None, None]

Sharding: data-parallel over B=8 across the 8 NeuronCores (one sample per
core). Per core the dynamic conv runs as K=39 fp16 matmuls: partition
q = n*3 + dyi holds the full image of band n shifted by dy (rows stored
258 wide with zero pad columns, so the dx shift is a free-dim offset);
the three dx matmuls accumulate in one PSUM bank. The shifted-replica
image is built once in SBUF (132 KB/partition) from a wide fp32->fp16
cast, so HBM traffic is just x in + weights in + out. The hypernetwork
runs with fp16 operands (host-cast weights) and fp32 PSUM. The
per-sample bias rides the PSUM->SBUF eviction.
"""

import numpy as np

import concourse.bacc as bacc
import concourse.bass as bass
import concourse.mybir as mybir
import concourse.tile as tile
from concourse.bass_utils import run_bass_kernel_spmd
from concourse.masks import make_identity

F32 = mybir.dt.float32
F16 = mybir.dt.float16

NB = 13          # bands
HH = WW = 256    # image
DE = 1024        # embed dim
DO = 128         # out channels
NCORES = 8

WPAD = WW + 2    # 258: row layout with a zero column at each end
GRP = 8          # psum banks in flight
OSTROWS = 8      # output rows per staging tile / output DMA (1 MB DMAs)


def _build_bass(repeat: int = 1, ablate: str = ""):
    # Bacc (not plain Bass): its finalize() runs generate_event_semaphores,
    # which splits multi-sem waits that TRN2 instruction structs can't hold.
    # repeat > 1 re-emits the main conv loop (benchmarking: slope between
    # repeat counts isolates device time from dispatch overhead).
    ab = set(ablate.split(",")) if ablate else set()
    nc = bacc.Bacc(target_bir_lowering=False, debug=False)

    # x is host-cast to fp16 and host-padded to 258-wide rows (zero col at
    # each end), so the im2col DMAs are fully contiguous per partition
    x_ext = nc.declare_dram_parameter("x", [NB, HH, WPAD], F16, isOutput=False)
    t_ext = nc.declare_dram_parameter("t_emb", [DE], F32, isOutput=False)
    wv_ext = nc.declare_dram_parameter("wv", [NB, DE], F32, isOutput=False)
    # w1/w2p/wb are host-cast to fp16; w2p/b2p host-permuted so generated
    # filter column c' = p*128 + d
    # w1p[p, k, m*128+s] = w1[k*128+p, m*128+s]; similarly w2p along k;
    # wbp[p, k, d] = wb[k*128+p, d]  (one contiguous DMA per weight)
    w1_ext = nc.declare_dram_parameter("w1p", [128, 8, 4 * DO], F16, isOutput=False)
    b1_ext = nc.declare_dram_parameter("b1", [4 * DO], F32, isOutput=False)
    w2p_ext = nc.declare_dram_parameter("w2pp", [128, 4, DO * 9], F16, isOutput=False)
    b2p_ext = nc.declare_dram_parameter("b2p", [DO * 9], F16, isOutput=False)
    wb_ext = nc.declare_dram_parameter("wbp", [128, 8, DO], F16, isOutput=False)
    bb_ext = nc.declare_dram_parameter("bb", [DO], F32, isOutput=False)
    out_ext = nc.declare_dram_parameter("out", [DO, HH, WW], F32, isOutput=True)

    with tile.TileContext(nc) as tc:
        with (
            tc.tile_pool(name="const", bufs=1) as const_pool,
            tc.tile_pool(name="resident", bufs=1) as res_pool,
            tc.tile_pool(name="hyp", bufs=1) as hyp_pool,
        ):
            # ---------------- hypernetwork (fp16 in / fp32 psum) ------------
            ident = const_pool.tile([128, 128], F32)
            make_identity(nc, ident[:])

            tT = hyp_pool.tile([128, 8], F32)   # t_emb[k*128+p] -> [p, k]
            nc.sync.dma_start(tT[:], t_ext.ap().rearrange("(k p) -> p k", p=128))
            b1T = hyp_pool.tile([128, 4], F32)
            nc.sync.dma_start(b1T[:], b1_ext.ap().rearrange("(m p) -> p m", p=128))
            bbT = hyp_pool.tile([128, 1], F32)
            nc.sync.dma_start(bbT[:], bb_ext.ap().rearrange("(p o) -> p o", o=1))
            b2pT = hyp_pool.tile([1, DO * 9], F16)
            nc.sync.dma_start(b2pT[:], b2p_ext.ap().rearrange("(o c) -> o c", o=1))
            ones1 = const_pool.tile([1, NB], F16)
            nc.vector.memset(ones1[:], 1.0)

            wv_t = hyp_pool.tile([NB, DE], F32)
            nc.sync.dma_start(wv_t[:], wv_ext.ap())

            w1p_t = hyp_pool.tile([128, 8, 4 * DO], F16)
            nc.sync.dma_start(w1p_t[:], w1_ext.ap())
            w2p_t = hyp_pool.tile([128, 4, DO * 9], F16)
            nc.sync.dma_start(w2p_t[:], w2p_ext.ap())
            wbp_t = hyp_pool.tile([128, 8, DO], F16)
            nc.sync.dma_start(wbp_t[:], wb_ext.ap())

            # ctxT[e, k, n] = wv[n, k*128+e] + t[k*128+e]   (fp16)
            ctxT = hyp_pool.tile([128, 8, NB], F16)
            with tc.tile_pool(name="tp_psum", bufs=2, space="PSUM") as tp_psum:
                # warm-up op: absorbs the identity-producer (Pool) semaphore
                # into the PE engine clock so later transposes carry a single
                # wait (the fused LDW struct has one wait slot).
                ps_warm = tp_psum.tile([1, 1], F32, tag="warm", bufs=1)
                nc.tensor.transpose(ps_warm[:], ident[:1, :1], ident[:1, :1])
                for k in range(8):
                    ps = tp_psum.tile([128, NB], F32, tag="tp")
                    nc.tensor.transpose(
                        ps[:], wv_t[:, k * 128:(k + 1) * 128], ident[:NB, :NB]
                    )
                    nc.vector.tensor_scalar_add(ctxT[:, k, :], ps[:], tT[:, k:k + 1])

                # sT[e, k] = sum_n ctxT[e, k, n]   (fp16 for the wb matmul)
                sT32 = hyp_pool.tile([128, 8, 1], F32)
                nc.vector.reduce_sum(sT32[:], ctxT[:], axis=mybir.AxisListType.X)
                sT = hyp_pool.tile([128, 8, 1], F16)
                nc.vector.tensor_copy(sT[:], sT32[:])

                # hidT[s, m, n] = silu(sum_e w1[e, m*128+s] * ctxT[e, n] + b1)
                hidT = hyp_pool.tile([128, 4, NB], F16)
                for m in range(4):
                    ps = tp_psum.tile([128, NB], F32, tag="hid")
                    for k in range(8):
                        nc.tensor.matmul(
                            ps[:], w1p_t[:, k, m * 128:(m + 1) * 128],
                            ctxT[:, k, :], start=(k == 0), stop=(k == 7)
                        )
                    nc.scalar.activation(
                        hidT[:, m, :], ps[:],
                        mybir.ActivationFunctionType.Silu, bias=b1T[:, m:m + 1],
                    )

                # wgen16[n, p*128+d] = hid @ w2p + b2p   (fp16)
                wgen16 = hyp_pool.tile([NB, DO * 9], F16)
                for j in range(3):  # 1152 = 3 * 384
                    ps = tp_psum.tile([NB, 384], F32, tag="wgen")
                    for k in range(4):
                        nc.tensor.matmul(
                            ps[:], hidT[:, k, :],
                            w2p_t[:, k, j * 384:(j + 1) * 384],
                            start=(k == 0), stop=False,
                        )
                    nc.tensor.matmul(
                        ps[:], ones1[:], b2pT[:, j * 384:(j + 1) * 384],
                        start=False, stop=True,
                    )
                    nc.vector.tensor_copy(wgen16[:, j * 384:(j + 1) * 384], ps[:])

                # bias[d] = sum_e s[e] * wb[e, d] + 13 * bb[d]
                bb13 = hyp_pool.tile([128, 1], F32)
                nc.vector.tensor_scalar_mul(bb13[:], bbT[:], float(NB))
                ps_b = tp_psum.tile([128, 1], F32, tag="bias", bufs=1)
                for k in range(8):
                    nc.tensor.matmul(
                        ps_b[:], wbp_t[:, k, :], sT[:, k, :],
                        start=(k == 0), stop=(k == 7)
                    )
                bias_sb = hyp_pool.tile([128, 1], F32)
                nc.scalar.activation(
                    bias_sb[:], ps_b[:],
                    mybir.ActivationFunctionType.Identity, bias=bb13[:],
                )

            # lhsT[dx][n*3+dyi, d] = wgen16[n, (dyi*3+dxi)*128 + d]
            # NOTE: only dim 0 of an SBUF AP crosses partitions, so one DMA
            # per (dx, dy): partition stride 3, offset dyi.
            lhsT = [
                hyp_pool.tile([3 * NB, DO], F16, tag=f"lhsT{i}", name=f"lhsT{i}")
                for i in range(3)
            ]
            wgen16_4d = wgen16[:].rearrange("n (dy dx d) -> n dy dx d", dy=3, dx=3)
            for dxi in range(3):
                lhsT_g = lhsT[dxi][:].rearrange("(n dy) d -> n dy d", dy=3)
                for dyi in range(3):
                    nc.sync.dma_start(
                        lhsT_g[:, dyi, :],
                        wgen16_4d[:, dyi, dxi, :],
                    )

            # ------- phase 0: build the dy-shifted fp16 image in SBUF -------
            # x39[n*3+dyi, r, 1+c] = x[n, r+dy, c]   (zeros at pads / edges)
            x39 = res_pool.tile([3 * NB, HH, WPAD], F16)
            # rows no DMA writes (image edge): zero across all partitions
            # first; the in-range dy groups' DMAs overwrite. Pad columns come
            # from the host-padded source rows.
            nc.gpsimd.memset(x39[:, 0:1, :], 0.0)
            nc.gpsimd.memset(x39[:, HH - 1:HH, :], 0.0)
            x39_g = x39[:].rearrange("(n dy) r w -> n dy r w", dy=3)
            for dyi, dy in enumerate((-1, 0, 1)):
                lo = max(0, -dy)
                hi = min(HH, HH - dy)
                nc.sync.dma_start(
                    x39_g[:, dyi, lo:hi, :],
                    x_ext.ap()[:, lo + dy:hi + dy, :],
                )

            # ---------------- main loop: dynamic conv -----------------------
            NPAIRS = HH // 2                    # 128 two-row pairs
            with (
                tc.tile_pool(name="ostage", bufs=4) as ostage_pool,
                tc.tile_pool(name="cpsum", bufs=GRP, space="PSUM") as cpsum_pool,
            ):
                for _rep in range(repeat):
                    for grp in range(NPAIRS // GRP):
                        psums = [
                            cpsum_pool.tile(
                                [DO, 2, WW], F32, tag="cps", name=f"cps{g}"
                            )
                            for g in range(GRP)
                        ]
                        # dx order (0, -1, +1): the dx=0 matmul reads no pad
                        # columns, keeping its wait count minimal.
                        dx_steps = (1,) if "mm1" in ab else (1, 0, 2)
                        for step, dxi in enumerate(dx_steps):
                            for g in range(GRP):
                                r0 = (grp * GRP + g) * 2
                                nc.tensor.matmul(
                                    psums[g][:],
                                    lhsT[dxi][:],
                                    x39[:, r0:r0 + 2, dxi:dxi + WW],
                                    start=(step == 0),
                                    stop=(step == len(dx_steps) - 1),
                                )
                        for ost_i in range(GRP * 2 // OSTROWS):
                            y0 = grp * GRP * 2 + ost_i * OSTROWS
                            ost = ostage_pool.tile([DO, OSTROWS, WW], F32, tag="ost")
                            for e in range(OSTROWS // 2):
                                g = ost_i * (OSTROWS // 2) + e
                                if g % 2 == 0:
                                    nc.scalar.activation(
                                        ost[:, 2 * e:2 * e + 2, :], psums[g][:],
                                        mybir.ActivationFunctionType.Identity,
                                        bias=bias_sb[:],
                                    )
                                else:
                                    nc.vector.tensor_scalar_add(
                                        ost[:, 2 * e:2 * e + 2, :], psums[g][:],
                                        bias_sb[:],
                                    )
                            # alternate the two HWDGE rings (SP / ACT)
                            dma_eng = nc.sync if (grp + ost_i) % 2 == 0 else nc.scalar
                            if "outslim" in ab:
                                dma_eng.dma_start(
                                    out_ext.ap()[:, y0:y0 + OSTROWS, 0:16],
                                    ost[:, :, 0:16],
                                )
                            else:
                                dma_eng.dma_start(
                                    out_ext.ap()[:, y0:y0 + OSTROWS, :], ost[:]
                                )
    if not nc.is_finalized():
        nc.finalize()
    return nc


_NC_CACHE = None


def _get_bass():
    global _NC_CACHE
    if _NC_CACHE is None:
        _NC_CACHE = _build_bass()
    return _NC_CACHE


def _prep_in_maps(inputs):
    x16 = np.asarray(inputs["x"], dtype=np.float32).astype(np.float16)
    x = np.zeros((x16.shape[0], NB, HH, WPAD), np.float16)
    x[:, :, :, 1:WW + 1] = x16
    t_emb = np.ascontiguousarray(np.asarray(inputs["t_emb"], dtype=np.float32))
    wv = np.ascontiguousarray(np.asarray(inputs["wv_embs"], dtype=np.float32))
    w1 = np.asarray(inputs["w1"], dtype=np.float32)
    b1 = np.ascontiguousarray(np.asarray(inputs["b1"], dtype=np.float32))
    w2 = np.asarray(inputs["w2"], dtype=np.float32)
    b2 = np.asarray(inputs["b2"], dtype=np.float32)
    wb = np.asarray(inputs["wb"], dtype=np.float32)
    bb = np.ascontiguousarray(np.asarray(inputs["bb"], dtype=np.float32))

    # permute filter columns: c = d*9 + p  ->  c' = p*128 + d; cast to fp16
    w2p = w2.reshape(4 * DO, DO, 9).transpose(0, 2, 1).reshape(4 * DO, DO * 9)
    w2pp = np.ascontiguousarray(
        w2p.reshape(4, 128, DO * 9).transpose(1, 0, 2)
    ).astype(np.float16)
    b2p = np.ascontiguousarray(b2.reshape(DO, 9).T.reshape(DO * 9)).astype(np.float16)
    w1p = np.ascontiguousarray(
        w1.reshape(8, 128, 4 * DO).transpose(1, 0, 2)
    ).astype(np.float16)
    wbp = np.ascontiguousarray(
        wb.reshape(8, 128, DO).transpose(1, 0, 2)
    ).astype(np.float16)

    return [
        {
            "x": x[b], "t_emb": t_emb[b], "wv": wv[b],
            "w1p": w1p, "b1": b1, "w2pp": w2pp, "b2p": b2p,
            "wbp": wbp, "bb": bb,
        }
        for b in range(NCORES)
    ]


def kernel(**inputs) -> np.ndarray:
    nc = _get_bass()
    in_maps = _prep_in_maps(inputs)
    res = run_bass_kernel_spmd(nc, in_maps, list(range(NCORES)))
    return np.stack([res.results[b]["out"] for b in range(NCORES)], axis=0)


if __name__ == "__main__":
    rng = np.random.default_rng(0)
    demo = {
        "x": rng.standard_normal((NCORES, NB, HH, WW), dtype=np.float32),
        "t_emb": rng.standard_normal((NCORES, DE), dtype=np.float32),
        "wv_embs": rng.standard_normal((NCORES, NB, DE), dtype=np.float32),
        "w1": rng.standard_normal((DE, 4 * DO), dtype=np.float32) * 0.02,
        "b1": np.zeros(4 * DO, np.float32),
        "w2": rng.standard_normal((4 * DO, DO * 9), dtype=np.float32) * 0.02,
        "b2": np.zeros(DO * 9, np.float32),
        "wb": rng.standard_normal((DE, DO), dtype=np.float32) * 0.02,
        "bb": np.zeros(DO, np.float32),
    }
    out = kernel(**demo)
    print("out", out.shape, out.dtype, float(np.abs(out).mean()))
